# revision 53
# baseline (speedup 1.0000x reference)
"""3-layer GCN (CircuitEncoder) on 8 TRN2 NeuronCores.

Sharding: batch dim (512 slices) -> 64 slices/core; weights + embedding table
replicated.  Norm factorization per slice:
    out[v] = dinv[v]*(sum_{e: col=v} g[row_e] + g[v]) + b,   g = dinv*(X@W)
so the per-edge path is a pure dma_gather + dma_scatter_add chain (self-loop
folded in by initializing the scatter accumulator AGG := G).

dma_scatter_add collapses duplicate indices within one call (one add per
destination per call, deterministic), but accumulates correctly across calls.
Edges are therefore grouped by occurrence-rank (computed on the host as pure
index marshalling): round r holds each destination's r-th edge, so indices
within a call are unique; rounds issue as sequential scatter calls.  deg is
computed with the same rounds scattering constant one-rows.

Host<->device I/O over this axon client's tunnel (~60 MB/s, saturated by
2 parallel streams) is the wall-clock bottleneck, so the runner is tuned
for wire traffic and overlap:
  - the jitted shard_map executable is built once and cached (the
    run_bass_kernel_spmd axon path re-traces + re-jits every call),
  - no host-built zero output buffers are shipped (the kernel writes
    every element of its outputs, so no init is needed),
  - idx tensors are stored 16-partition-wide and replicated to 128
    partitions on-device (8x less h2d), h1 = emb @ W1 is host-computed,
  - the final layer is quantized on-device to 7-bit codes (clamped round
    of x*127/max) packed 8-into-7-bytes via u32 lane ops, with a per-node
    f32 scale; the host unpacks and dequantizes (~1% error in quadrature),
  - the batch is processed as four quarter-invocations of one NEFF so
    the d2h fetch of chunk h overlaps the execution of chunk h+1, with
    shared weight tensors uploaded once as device-resident arrays.
"""

import sys

sys.path.insert(0, "/opt/trn_rl_repo")

import numpy as np
import ml_dtypes

import concourse.bacc as bacc
import concourse.bass as bass
import concourse.mybir as mybir
import concourse.tile as tile
from concourse import library_config

NCORES = 8
B, E, NPN, D = 512, 2048, 1024, 128
SLICES = B // NCORES          # 64 slices per core
RSP = 16                      # slices per region (scatter idx < 16384 int16)
NCALLS = 4                    # quarter-batch invocations of the same NEFF
SL_C = SLICES // NCALLS       # 16 slices per core per invocation
NREG = SL_C // RSP            # 1 region per invocation
NODES_R = RSP * NPN           # 16384 rows per region
NJUNK = 128                   # junk rows for padded scatter slots
N_C = SL_C * NPN              # 32768 nodes per core per invocation
BF = mybir.dt.bfloat16
F32 = mybir.dt.float32
I16 = mybir.dt.int16

PKW = D // 8 * 7              # 112 packed bytes per output row (7-bit codes)
ABLK = 2048                   # nodes per compute half-block
DBLK = 4096                   # nodes per DMA block (one DMA, two halves)
NAB = NODES_R // DBLK         # 4 DMA blocks per region

# rank-round call capacities (per 16-slice region, 32768 edges).
# counts ~ 16384*P(Pois(2)>=r+1); caps = count + 6*sqrt + slack, %16,
# each <= 8064 (SWDGE ring: m2s = n/8+1 <= 1024).  The last call takes all
# ranks >= len(CAPS)-1 (duplicate collapse eats ~0.4 expected edges).
CAPS = [7456, 7456, 7456, 2656, 5632, 2688, 1152, 448, 176, 80, 48, 32, 32]
# round id per call (r0 and r1 split into two calls each)
CALL_ROUND = [0, 0, 1, 1, 2, 3, 4, 5, 6, 7, 8, 9, 10]
LPAD = sum(CAPS)              # 35312 padded slots per region
MAXCALL = max(CAPS)

SHARED_IN = ("h1", "W1", "W2", "b")


def _build(compile_nc=True):
    nc = bacc.Bacc(None, target_bir_lowering=False)

    # h1 = emb @ W1 is slice-independent; the host computes it exactly in
    # f32 and ships it, removing emb/W1 uploads + the on-device transpose
    # and warm-up matmuls from chunk 0's critical path
    h1p = nc.declare_dram_parameter("h1", [NPN, D], BF, isOutput=False)
    Ws = {i: nc.declare_dram_parameter(f"W{i}", [D, D], BF, isOutput=False) for i in (1, 2)}
    bsrc = nc.declare_dram_parameter("b", [1, 3 * D], F32, isOutput=False)
    idxR = [nc.declare_dram_parameter(f"idxR{r}", [16, LPAD // 16], I16, isOutput=False) for r in range(NREG)]
    idxC = [nc.declare_dram_parameter(f"idxC{r}", [16, LPAD // 16], I16, isOutput=False) for r in range(NREG)]
    # final layer ships quantized: 128 7-bit codes packed into 112 bytes +
    # the per-node bf16 scale's 2 raw bytes appended per row (one contiguous
    # d2h transfer per shard)
    outq = nc.declare_dram_parameter("outq", [N_C, PKW + 2], mybir.dt.uint8, isOutput=True)

    Gd = [nc.dram_tensor(f"Gd{r}", [NODES_R, D], BF) for r in range(NREG)]
    AGG = [nc.dram_tensor(f"AGG{r}", [NODES_R + NJUNK, D], BF) for r in range(NREG)]
    X2 = [nc.dram_tensor(f"X2_{r}", [NODES_R, D], BF) for r in range(NREG)]
    X3 = [nc.dram_tensor(f"X3_{r}", [NODES_R, D], BF) for r in range(NREG)]
    DINV = [nc.dram_tensor(f"DINV{r}", [NODES_R, D], BF) for r in range(NREG)]

    call_off = np.cumsum([0] + CAPS).tolist()

    with tile.TileContext(nc) as tc:
        with (
            tc.tile_pool(name="const", bufs=1) as cpool,
            tc.tile_pool(name="idx", bufs=2) as ipool,
            tc.tile_pool(name="msg", bufs=2) as mpool,
            tc.tile_pool(name="work", bufs=2) as apool,
            tc.tile_pool(name="quant", bufs=1) as qpool,
            tc.tile_pool(name="psum", bufs=2, space="PSUM") as ppool,
        ):
            nc.gpsimd.load_library(library_config.mlp)

            # ---- constants ----
            wbf = {}
            for i in (1, 2):
                wb = cpool.tile([128, D], BF, tag=f"wb{i}")
                nc.sync.dma_start(wb[:], Ws[i][:, :])
                wbf[i] = wb
            # bias [1, 3*D] -> broadcast to all 128 partitions via a
            # 1-contraction fp32 matmul (outer product with a ones column)
            b1p = cpool.tile([1, 3 * D], F32)
            nc.sync.dma_start(b1p[:], bsrc[:, :])
            ones1 = cpool.tile([1, 128], F32)
            nc.vector.memset(ones1[:], 1.0)
            psb = ppool.tile([128, ABLK], F32, tag="ps")
            nc.tensor.matmul(
                psb[:, 0:3 * D], lhsT=ones1[:], rhs=b1p[:], start=True, stop=True
            )
            bias_sb = cpool.tile([128, 3, D], F32)
            nc.vector.tensor_copy(
                out=bias_sb[:],
                in_=psb[:, 0:3 * D].rearrange("p (l d) -> p l d", d=D),
            )

            # h1 = emb @ W1 (host-computed, shared by all slices),
            # node-major [p, c, f]
            h1sb = cpool.tile([128, 8, D], BF)
            nc.sync.dma_start(h1sb[:], h1p.rearrange("(c p) d -> p c d", p=128))

            ones = cpool.tile([128, MAXCALL // 128 + 1, D], BF)
            nc.vector.memset(ones[:], 1.0)

            def load_idx(param, r):
                # 16-partition-wide in DRAM; replicate to 128 partitions
                # on-device (the gather/scatter idx AP wants 8x replication)
                t = ipool.tile([128, LPAD // 16], I16, tag="idx")
                for g in range(8):
                    eng = nc.sync if g % 2 == 0 else nc.scalar
                    eng.dma_start(t[g * 16:(g + 1) * 16, :], param[:, :])
                return t

            def b_calls(r, idxC_t, src_msgs=None, idxR_t=None, Gsrc=None):
                """Issue the per-region round calls: optional gather into msg
                tiles then scatter-add into AGG[r]."""
                for c, cap in enumerate(CAPS):
                    o = call_off[c]
                    if Gsrc is not None:
                        msg = mpool.tile([128, MAXCALL // 128 + 1, D], BF, tag="msg")
                        nc.gpsimd.dma_gather(
                            msg[:, : (cap + 127) // 128, :],
                            Gsrc[:, :],
                            idxR_t[:, o // 16:(o + cap) // 16],
                            cap,
                            cap,
                            D,
                            single_packet=False,
                        )
                        src = msg
                    else:
                        src = ones
                    nc.gpsimd.dma_scatter_add(
                        AGG[r][:, :],
                        src[:, : (cap + 127) // 128, :],
                        idxC_t[:, o // 16:(o + cap) // 16],
                        cap,
                        cap,
                        D,
                        single_packet=False,
                    )

            # ---- degree (scatter ones), then dinv = 1/sqrt(deg) ----
            for r in range(NREG):
                idxC_t = load_idx(idxC[r], r)
                for blk in range(NODES_R // ABLK):  # init deg = 1 (self-loop)
                    eng = nc.sync if blk % 2 == 0 else nc.scalar
                    eng.dma_start(
                        AGG[r][blk * ABLK:(blk + 1) * ABLK, :].rearrange(
                            "(c p) d -> p c d", p=128
                        ),
                        ones[:, : ABLK // 128, :],
                    )
                b_calls(r, idxC_t)
                for blk in range(NAB):
                    eng = nc.sync if blk % 2 == 0 else nc.scalar
                    r0 = blk * DBLK
                    deg_t = apool.tile([128, DBLK // 128, D], BF, tag="cin")
                    eng.dma_start(
                        deg_t[:],
                        AGG[r][r0:r0 + DBLK, :].rearrange(
                            "(c p) d -> p c d", p=128
                        ),
                    )
                    dinv_t = apool.tile([128, DBLK // 128, D], BF, tag="cout")
                    for h in range(2):
                        sq_t = apool.tile([128, ABLK // 128, D], BF, tag="ct1")
                        nc.scalar.activation(
                            out=sq_t[:],
                            in_=deg_t[:, h * (ABLK // 128):(h + 1) * (ABLK // 128), :],
                            func=mybir.ActivationFunctionType.Sqrt,
                        )
                        with nc.allow_low_precision(reason="bf16 gcn kernel"):
                            nc.vector.reciprocal(
                                out=dinv_t[:, h * (ABLK // 128):(h + 1) * (ABLK // 128), :],
                                in_=sq_t[:],
                            )
                    eng.dma_start(
                        DINV[r][r0:r0 + DBLK, :].rearrange(
                            "(c p) d -> p c d", p=128
                        ),
                        dinv_t[:],
                    )

            # ---- 3 GCN layers ----
            for l in range(3):
                for r in range(NREG):
                    # A-pass: G = dinv * (X @ W); AGG := G
                    if l == 0:
                        for s in range(RSP):
                            eng = nc.sync if s % 2 == 0 else nc.scalar
                            r0 = s * NPN
                            dinv_t = apool.tile([128, 8, D], BF, tag="adinv")
                            eng.dma_start(
                                dinv_t[:],
                                DINV[r][r0:r0 + NPN, :].rearrange(
                                    "(c p) d -> p c d", p=128
                                ),
                            )
                            g_t = apool.tile([128, 8, D], BF, tag="agout")
                            nc.vector.tensor_tensor(
                                out=g_t[:], in0=h1sb[:], in1=dinv_t[:],
                                op=mybir.AluOpType.mult,
                            )
                            for dst in (Gd[r], AGG[r]):
                                eng.dma_start(
                                    dst[r0:r0 + NPN, :].rearrange(
                                        "(c p) d -> p c d", p=128
                                    ),
                                    g_t[:],
                                )
                    else:
                        Xsrc = X2[r] if l == 1 else X3[r]
                        for blk in range(NAB):
                            eng = nc.sync if blk % 2 == 0 else nc.scalar
                            r0 = blk * DBLK
                            xT = apool.tile([128, DBLK], BF, tag="axT")
                            nc.sync.dma_start_transpose(xT[:], Xsrc[r0:r0 + DBLK, :])
                            dinv_t = apool.tile([128, DBLK // 128, D], BF, tag="adinv")
                            eng.dma_start(
                                dinv_t[:],
                                DINV[r][r0:r0 + DBLK, :].rearrange(
                                    "(c p) d -> p c d", p=128
                                ),
                            )
                            g_t = apool.tile([128, DBLK // 128, D], BF, tag="agout")
                            for h in range(2):
                                ps = ppool.tile([128, ABLK], F32, tag="ps")
                                for c in range(ABLK // 128):
                                    nc.tensor.matmul(
                                        ps[:, c * D:(c + 1) * D],
                                        lhsT=xT[:, h * ABLK + c * 128:h * ABLK + (c + 1) * 128],
                                        rhs=wbf[l][:],
                                        start=True,
                                        stop=True,
                                    )
                                hc = ABLK // 128
                                nc.vector.tensor_tensor(
                                    out=g_t[:, h * hc:(h + 1) * hc, :],
                                    in0=ps[:].rearrange("p (c d) -> p c d", d=D),
                                    in1=dinv_t[:, h * hc:(h + 1) * hc, :],
                                    op=mybir.AluOpType.mult,
                                )
                            for dst in (Gd[r], AGG[r]):
                                eng.dma_start(
                                    dst[r0:r0 + DBLK, :].rearrange(
                                        "(c p) d -> p c d", p=128
                                    ),
                                    g_t[:],
                                )

                for r in range(NREG):
                    # B-pass: gather by src node, rank-round scatter-adds
                    idxR_t = load_idx(idxR[r], r)
                    idxC_t = load_idx(idxC[r], r)
                    b_calls(r, idxC_t, idxR_t=idxR_t, Gsrc=Gd[r])

                for r in range(NREG):
                    # C-pass: X_next = relu(dinv * AGG + b)
                    for blk in range(NAB):
                        eng = nc.sync if blk % 2 == 0 else nc.scalar
                        r0 = blk * DBLK
                        hc = ABLK // 128
                        agg_t = apool.tile([128, DBLK // 128, D], BF, tag="cin")
                        eng.dma_start(
                            agg_t[:],
                            AGG[r][r0:r0 + DBLK, :].rearrange(
                                "(c p) d -> p c d", p=128
                            ),
                        )
                        dinv_t = apool.tile([128, DBLK // 128, D], BF, tag="adinv")
                        eng.dma_start(
                            dinv_t[:],
                            DINV[r][r0:r0 + DBLK, :].rearrange(
                                "(c p) d -> p c d", p=128
                            ),
                        )
                        xo = apool.tile(
                            [128, DBLK // 128, D], BF if l < 2 else F32, tag="cout"
                        )
                        for h in range(2):
                            t1 = apool.tile([128, hc, D], BF, tag="ct1")
                            nc.vector.tensor_tensor(
                                out=t1[:],
                                in0=agg_t[:, h * hc:(h + 1) * hc, :],
                                in1=dinv_t[:, h * hc:(h + 1) * hc, :],
                                op=mybir.AluOpType.mult,
                            )
                            t2 = apool.tile([128, hc, D], F32, tag="coutf")
                            nc.vector.tensor_tensor(
                                out=t2[:],
                                in0=t1[:],
                                in1=bias_sb[:, l:l + 1, :].broadcast_to(
                                    [128, hc, D]
                                ),
                                op=mybir.AluOpType.add,
                            )
                            nc.scalar.activation(
                                out=xo[:, h * hc:(h + 1) * hc, :], in_=t2[:],
                                func=mybir.ActivationFunctionType.Relu,
                            )
                        if l < 2:
                            Xdst = X2[r] if l == 0 else X3[r]
                            eng.dma_start(
                                Xdst[r0:r0 + DBLK, :].rearrange(
                                    "(c p) d -> p c d", p=128
                                ),
                                xo[:],
                            )
                        else:
                            # quantize: q = round(xo * 127/max_d) CLAMPED to
                            # 127 (vector.reciprocal is approximate: when
                            # 1/max rounds up, the max element's code lands
                            # on 128, whose bit 7 would corrupt the 7-bit
                            # pack); scale = max_d
                            nb = DBLK // 128
                            mx = qpool.tile([128, nb], F32, tag="qmx")
                            nc.vector.tensor_reduce(
                                out=mx[:], in_=xo[:],
                                axis=mybir.AxisListType.X,
                                op=mybir.AluOpType.max,
                            )
                            rs = qpool.tile([128, nb], F32, tag="qrs")
                            nc.vector.tensor_scalar(
                                out=rs[:], in0=mx[:], scalar1=1e-20,
                                scalar2=None, op0=mybir.AluOpType.max,
                            )
                            nc.vector.reciprocal(out=rs[:], in_=rs[:])
                            nc.vector.tensor_scalar(
                                out=rs[:], in0=rs[:], scalar1=127.0,
                                scalar2=None, op0=mybir.AluOpType.mult,
                            )
                            q = qpool.tile(
                                [128, nb, D], mybir.dt.uint8, tag="qout"
                            )
                            for h in range(2):
                                qf = apool.tile([128, hc, D], F32, tag="coutf")
                                nc.vector.tensor_tensor(
                                    out=qf[:],
                                    in0=xo[:, h * hc:(h + 1) * hc, :],
                                    in1=rs[:, h * hc:(h + 1) * hc, None]
                                    .broadcast_to([128, hc, D]),
                                    op=mybir.AluOpType.mult,
                                )
                                nc.vector.tensor_scalar(
                                    out=q[:, h * hc:(h + 1) * hc, :],
                                    in0=qf[:], scalar1=0.5, scalar2=127.0,
                                    op0=mybir.AluOpType.add,
                                    op1=mybir.AluOpType.min,
                                )
                            # pack 4 codes per u32 lane to 28 bits, then
                            # weave pairs/quads — all accesses u32-aligned.
                            # W = sum_k ((Q >> k) & (0x7F << 7k))
                            U32 = mybir.dt.uint32
                            shr = mybir.AluOpType.logical_shift_right
                            shl = mybir.AluOpType.logical_shift_left
                            band = mybir.AluOpType.bitwise_and
                            bor = mybir.AluOpType.bitwise_or
                            qU = q[:].bitcast(U32)            # [128, nb, 32]
                            W = qpool.tile([128, nb, 32], U32, tag="qW")
                            tm = qpool.tile([128, nb, 32], U32, tag="qT")
                            nc.vector.tensor_scalar(
                                out=W[:], in0=qU, scalar1=0x7F,
                                scalar2=None, op0=band,
                            )
                            for k in (1, 2, 3):
                                nc.vector.tensor_scalar(
                                    out=tm[:], in0=qU, scalar1=k,
                                    scalar2=0x7F << (7 * k),
                                    op0=shr, op1=band,
                                )
                                nc.vector.tensor_tensor(
                                    out=W[:], in0=W[:], in1=tm[:], op=bor
                                )
                            # pairs: P0 = We | Wo<<28 (4B), P1 = Wo>>4 (3B)
                            Wg = W[:].rearrange("p c (g t) -> p c g t", t=2)
                            We, Wo = Wg[:, :, :, 0], Wg[:, :, :, 1]
                            pk = qpool.tile([128, nb, 28], U32, tag="qpk")
                            t16 = qpool.tile([128, nb, 16], U32, tag="qt16")
                            nc.vector.tensor_scalar(
                                out=t16[:], in0=Wo, scalar1=28,
                                scalar2=None, op0=shl,
                            )
                            nc.vector.tensor_tensor(
                                out=pk[:, :, 0:16], in0=We, in1=t16[:], op=bor
                            )
                            P1 = qpool.tile([128, nb, 16], U32, tag="qP1")
                            nc.vector.tensor_scalar(
                                out=P1[:], in0=Wo, scalar1=4,
                                scalar2=None, op0=shr,
                            )
                            # quads of 3-byte P1 -> 3 u32: PP0=l0|l1<<24,
                            # PP1=l1>>8|l2<<16, PP2=l2>>16|l3<<8
                            Pg = P1[:].rearrange("p c (g t) -> p c g t", t=4)
                            pp = pk[:, :, 16:28].rearrange(
                                "p c (g t) -> p c g t", t=3
                            )
                            tA = qpool.tile([128, nb, 4], U32, tag="qtA")
                            tB = qpool.tile([128, nb, 4], U32, tag="qtB")
                            for t, (la, sa, lb, sb) in enumerate(
                                ((0, 0, 1, 24), (1, -8, 2, 16), (2, -16, 3, 8))
                            ):
                                nc.vector.tensor_scalar(
                                    out=tA[:], in0=Pg[:, :, :, la],
                                    scalar1=-sa, scalar2=None,
                                    op0=shr if sa < 0 else shl,
                                )
                                nc.vector.tensor_scalar(
                                    out=tB[:], in0=Pg[:, :, :, lb],
                                    scalar1=sb, scalar2=None, op0=shl,
                                )
                                nc.vector.tensor_tensor(
                                    out=pp[:, :, :, t], in0=tA[:], in1=tB[:],
                                    op=bor,
                                )
                            eng.dma_start(
                                outq[
                                    r * NODES_R + r0:r * NODES_R + r0 + DBLK,
                                    0:PKW,
                                ].rearrange("(c p) d -> p c d", p=128),
                                pk[:].bitcast(mybir.dt.uint8),
                            )
                            mxb = qpool.tile([128, nb], BF, tag="qmxb")
                            nc.vector.tensor_copy(out=mxb[:], in_=mx[:])
                            eng.dma_start(
                                outq[
                                    r * NODES_R + r0:r * NODES_R + r0 + DBLK,
                                    PKW:PKW + 2,
                                ].rearrange("(c p) d -> p c d", p=128),
                                mxb[:].bitcast(mybir.dt.uint8).rearrange(
                                    "p (c b) -> p c b", b=2
                                ),
                            )
    if compile_nc:
        nc.compile()
    return nc


def _prep_idx(edges_block):
    """edges_block [SL_C, 2, 2048] int -> per-region padded wrapped idx arrays.

    Host work is pure index marshalling: stable-sort edge ids by destination
    to find each edge's occurrence rank, place rank-r edges into round r's
    static slot range, pad gathers with 0 and scatters with junk rows.
    """
    idxRs, idxCs = [], []
    call_off = np.cumsum([0] + CAPS)
    for r in range(NREG):
        sl = edges_block[r * RSP:(r + 1) * RSP]         # [16, 2, 2048]
        offs = (np.arange(RSP, dtype=np.int64) * NPN)[:, None]
        row = (sl[:, 0, :] + offs).reshape(-1)          # [32768]
        col = (sl[:, 1, :] + offs).reshape(-1)
        ne = col.shape[0]
        order = np.lexsort((np.arange(ne), col))        # stable by col
        sc = col[order]
        first = np.ones(ne, dtype=bool)
        first[1:] = sc[1:] != sc[:-1]
        run_id = np.cumsum(first) - 1
        run_start = np.nonzero(first)[0]
        rank = np.arange(ne) - run_start[run_id]        # occurrence rank
        rank_of_edge = np.empty(ne, dtype=np.int64)
        rank_of_edge[order] = rank
        rank_of_edge = np.minimum(rank_of_edge, CALL_ROUND[-1])

        rowp = np.zeros(LPAD, dtype=np.int16)
        colp = np.empty(LPAD, dtype=np.int16)
        junk = NODES_R + (np.arange(LPAD) % NJUNK)
        colp[:] = junk.astype(np.int16)
        for c, cap in enumerate(CAPS):
            rd = CALL_ROUND[c]
            e_ids = np.nonzero(rank_of_edge == rd)[0]
            if CALL_ROUND.count(rd) > 1:
                prev = sum(CAPS[j] for j in range(c) if CALL_ROUND[j] == rd)
                e_ids = e_ids[prev:prev + cap]
            if len(e_ids) > cap:
                # astronomically rare; drop the tail edges (error ~1e-4)
                e_ids = e_ids[:cap]
            o = call_off[c]
            rowp[o:o + len(e_ids)] = row[e_ids]
            colp[o:o + len(e_ids)] = col[e_ids]

        def wrap(a):
            return np.ascontiguousarray(a.reshape(LPAD // 16, 16).T)

        idxRs.append(wrap(rowp))
        idxCs.append(wrap(colp))
    return idxRs, idxCs


_NC_CACHE = None


def _get_nc():
    global _NC_CACHE
    if _NC_CACHE is None:
        _NC_CACHE = _build()
    return _NC_CACHE


# ---------------------------------------------------------------------------
# Runner: cached jitted shard_map over 8 cores via the bass_exec custom call
# (the same execution stack run_bass_kernel_spmd uses under axon, minus the
# per-call re-trace/re-jit and minus host-built zero output buffers).
# ---------------------------------------------------------------------------

_RUNNER_CACHE = None


def _get_runner():
    global _RUNNER_CACHE
    if _RUNNER_CACHE is not None:
        return _RUNNER_CACHE

    import jax
    from jax.sharding import Mesh, PartitionSpec, NamedSharding
    from jax.experimental.shard_map import shard_map
    from concourse.bass2jax import (
        install_neuronx_cc_hook,
        _bass_exec_p,
        partition_id_tensor,
    )

    nc = _get_nc()
    install_neuronx_cc_hook()

    partition_name = nc.partition_id_tensor.name if nc.partition_id_tensor else None
    in_names, out_names, out_avals = [], [], []
    in_shapes = {}
    for alloc in nc.m.functions[0].allocations:
        if not isinstance(alloc, mybir.MemoryLocationSet):
            continue
        name = alloc.memorylocations[0].name
        if alloc.kind == "ExternalInput":
            if name != partition_name:
                in_names.append(name)
                in_shapes[name] = (
                    tuple(alloc.tensor_shape), mybir.dt.np(alloc.dtype)
                )
        elif alloc.kind == "ExternalOutput":
            out_names.append(name)
            out_avals.append(
                jax.core.ShapedArray(
                    tuple(alloc.tensor_shape), mybir.dt.np(alloc.dtype)
                )
            )
    bind_in_names = tuple(in_names + ([partition_name] if partition_name else []))

    def _body(*args):
        operands = list(args)
        if partition_name is not None:
            operands.append(partition_id_tensor())
        return tuple(
            _bass_exec_p.bind(
                *operands,
                out_avals=tuple(out_avals),
                in_names=bind_in_names,
                out_names=tuple(out_names),
                lowering_input_output_aliases=(),
                sim_require_finite=True,
                sim_require_nnan=True,
                nc=nc,
            )
        )

    devices = jax.devices()[:NCORES]
    mesh = Mesh(np.asarray(devices), ("core",))
    # slice-independent inputs are replicated (P()) instead of 8x-concatenated
    # P("core") shards — the client then ships one copy over the tunnel
    rep_sh = NamedSharding(mesh, PartitionSpec())
    in_specs = tuple(
        PartitionSpec() if name in SHARED_IN else PartitionSpec("core")
        for name in in_names
    )
    sharded = jax.jit(
        shard_map(
            _body,
            mesh=mesh,
            in_specs=in_specs,
            out_specs=(PartitionSpec("core"),) * len(out_names),
            check_rep=False,
        ),
        keep_unused=True,
    )

    # dbg_addr (if present) is an unused 8-byte ExternalInput; bind zeros
    # as uint32[1,2] — x64-off canonicalization would shrink uint64 to 4B.
    fillers = {}
    if nc.dbg_addr is not None:
        if nc.dbg_callbacks:
            raise RuntimeError("dbg_callbacks unsupported on the axon client")
        fillers[nc.dbg_addr.name] = np.zeros((1, 2), np.uint32)
    for name, (shape, dtype) in in_shapes.items():
        if name not in SHARED_IN and not (
            name.startswith("idxR") or name.startswith("idxC")
        ):
            fillers.setdefault(name, np.zeros(shape, dtype))

    _RUNNER_CACHE = (sharded, in_names, out_names, fillers, rep_sh)
    return _RUNNER_CACHE


def kernel(edge_index, qubit_embeddings, W1, b1, W2, b2, W3, b3):
    import jax
    from concurrent.futures import ThreadPoolExecutor

    sharded, in_names, out_names, fillers, rep_sh = _get_runner()

    edge_index = np.asarray(edge_index).astype(np.int64)
    emb_f = np.asarray(qubit_embeddings, dtype=np.float32)
    Wf = [np.asarray(w, dtype=np.float32) for w in (W1, W2, W3)]
    h1 = (emb_f @ Wf[0]).astype(ml_dtypes.bfloat16)
    Ws = [w.astype(ml_dtypes.bfloat16) for w in Wf[1:]]
    bs = [np.asarray(b, dtype=np.float32) for b in (b1, b2, b3)]
    bflat = np.concatenate(bs)[None, :]           # [1, 3*D]

    # shared (slice-independent) inputs: upload once (replicated sharding,
    # a single copy over the tunnel), reuse in every chunk invocation
    shared_np = {"h1": h1, "W1": Ws[0], "W2": Ws[1], "b": bflat}
    shared_dev = {
        name: jax.device_put(arr, rep_sh) for name, arr in shared_np.items()
    }

    # fetch the per-device shards concurrently (two streams saturate the
    # tunnel; chunk h downloads while chunk h+1 still executes), unpack the
    # 7-bit codes and dequantize: full[v, :] = v[v, :] * (scale[v] / 127)
    full = np.empty((B * NPN, D), np.float32)

    def _fetch(task):
        h, qsh = task
        core = qsh.index[0].start // N_C
        r0 = core * SLICES * NPN + h * N_C
        rows = slice(r0, r0 + N_C)
        raw = np.asarray(qsh.data)
        scale = (
            raw[:, PKW:PKW + 2].copy().view(ml_dtypes.bfloat16)
            .astype(np.float32).ravel()
        )
        # inverse of the device u32 packing: 28 u32 -> 32 28-bit W -> codes
        u = np.ascontiguousarray(raw[:, :PKW]).view(np.uint32)   # [n, 28]
        n = u.shape[0]
        P0, PPf = u[:, 0:16], u[:, 16:28]
        PP0, PP1, PP2 = PPf[:, 0::3], PPf[:, 1::3], PPf[:, 2::3]
        P1 = np.empty((n, 16), np.uint32)
        P1[:, 0::4] = PP0 & 0xFFFFFF
        P1[:, 1::4] = (PP0 >> 24) | ((PP1 & 0xFFFF) << 8)
        P1[:, 2::4] = (PP1 >> 16) | ((PP2 & 0xFF) << 16)
        P1[:, 3::4] = PP2 >> 8
        W = np.empty((n, 32), np.uint32)
        W[:, 0::2] = P0 & 0x0FFFFFFF
        W[:, 1::2] = (P1 << 4) | (P0 >> 28)
        blk = full[rows]
        for k in range(4):
            blk[:, k::4] = (W >> np.uint32(7 * k)) & np.uint32(0x7F)
        blk *= (scale * np.float32(1 / 127))[:, None]

    qi = out_names.index("outq")
    ex = ThreadPoolExecutor(NCORES)
    futs = []
    for h in range(NCALLS):
        per_core = []
        for c in range(NCORES):
            lo = c * SLICES + h * SL_C
            idxRs, idxCs = _prep_idx(edge_index[lo:lo + SL_C])
            m = {}
            for r in range(NREG):
                m[f"idxR{r}"] = idxRs[r]
                m[f"idxC{r}"] = idxCs[r]
            per_core.append(m)
        args = []
        for name in in_names:
            if name in shared_dev:
                args.append(shared_dev[name])
            else:
                args.append(
                    np.concatenate(
                        [np.asarray(m.get(name, fillers.get(name)))
                         for m in per_core],
                        axis=0,
                    )
                )
        outs_g = sharded(*args)          # async dispatch
        q_g = outs_g[qi]
        q_shards = sorted(q_g.addressable_shards, key=lambda s: s.index[0].start)
        futs += [ex.submit(_fetch, (h, q)) for q in q_shards]
    for f in futs:
        f.result()
    ex.shutdown()
    return full
